# revision 2
# baseline (speedup 1.0000x reference)
"""Trainium2 Bass kernel for nn_DeepDendriticEncoder.

Computes, for every sliding window n of length 256 over x[0:500000]:
    h1 = relu(X @ W1.T); h2 = relu(h1 @ W2.T); h3 = relu(h2 @ W3.T)
    I[n] = 2 * max_k h3[n, k]
on 8 NeuronCores (window axis sharded, W-1 halo on x), then finishes the
tiny LIF latency / argmin chain on host with jax ops that replicate the
reference.

Device strategy per core (Hankel windows never materialized in DRAM):
  - for each block of 512 windows, DMA a "diagonal" tile D[i, c] =
    x[base + i + c] (128 x 640, overlapping strided read straight from HBM)
  - conv-as-matmul: h1 = W1a.T @ D[:, 0:512] + W1b.T @ D[:, 128:640]
    accumulated in PSUM (contraction = tap index, 2 x 128)
  - h2 via W2.T stationary; layer 3 swaps operands (relu(h2) chunks
    stationary, W3.T moving) so h3 lands [window, k3] and the max over
    k3 is a free-axis DVE reduce - no cross-partition reduction needed.
  - per-core I values accumulate in SBUF, one DMA out at the end.

Matmul inputs run in bf16 (fp32 PSUM accumulation). The downstream
consumers are cliff functions with enormous margins for this problem
family (spike threshold I>1, integer step counts, argmax gaps), so
bf16-level error (~1e-2 relative) is far below every decision margin;
the reported winner/latency values are recomputed on host in f32.
"""

import sys

for _p in ("/opt/trn_rl_repo",):
    if _p not in sys.path:
        sys.path.insert(0, _p)

import numpy as np

# ---- problem constants (match reference.py; hardcoded by contract) ----
T = 500000
W_WIN = 256
K = 128
DT = 0.01
TAU = 0.05
DECAY = 1.0 - DT / TAU  # 0.8
MAX_STEPS = 200000
N = T - W_WIN + 1  # 499745

NCORES = 8
NPC = (N + NCORES - 1) // NCORES  # 62469 windows per core (last core fewer)
BLK = 512
NBLK = (NPC + BLK - 1) // BLK  # 123
CAP = NBLK * BLK  # 62976 windows computed per core (incl. padding)
XSH = CAP + 2 * K  # 63232 x-shard length (diag tile needs base+766)

USE_BF16 = True

_compiled = None


def _build():
    """Build + compile the SPMD Bass program once per process."""
    import concourse.bass as bass
    import concourse.tile as tile
    from concourse import bacc, mybir

    f32 = mybir.dt.float32
    dtm = mybir.dt.bfloat16 if USE_BF16 else f32
    nc = bacc.Bacc("TRN2", target_bir_lowering=False)

    xs = nc.dram_tensor("xs", [XSH], dtm, kind="ExternalInput")
    w1t = nc.dram_tensor("w1t", [128, 256], dtm, kind="ExternalInput")
    w2t = nc.dram_tensor("w2t", [128, 64], dtm, kind="ExternalInput")
    w3t = nc.dram_tensor("w3t", [64, 32], dtm, kind="ExternalInput")
    iout = nc.dram_tensor("iout", [128, NBLK * 4], f32, kind="ExternalOutput")

    with tile.TileContext(nc) as tc:
        with (
            tc.tile_pool(name="const", bufs=1) as cpool,
            tc.tile_pool(name="diag", bufs=3) as dpool,
            tc.tile_pool(name="acts", bufs=3) as rpool,
            tc.tile_pool(name="iacc", bufs=1) as ipool,
            tc.tile_pool(name="ps1", bufs=2, space="PSUM") as ps1,
            tc.tile_pool(name="ps2", bufs=2, space="PSUM") as ps2,
            tc.tile_pool(name="ps3", bufs=2, space="PSUM") as ps3,
        ):
            w1s = cpool.tile([128, 256], dtm)
            nc.sync.dma_start(w1s[:], w1t[:])
            w2s = cpool.tile([128, 64], dtm)
            nc.sync.dma_start(w2s[:], w2t[:])
            w3s = cpool.tile([64, 32], dtm)
            nc.sync.dma_start(w3s[:], w3t[:])
            isb = ipool.tile([128, NBLK * 4], f32)

            for blk in range(NBLK):
                base = blk * BLK
                d = dpool.tile([128, BLK + 128], dtm, tag="d")
                nc.sync.dma_start(
                    d[:], bass.AP(xs, base, [[1, 128], [1, BLK + 128]])
                )

                p1 = ps1.tile([128, BLK], f32, tag="p1")
                nc.tensor.matmul(
                    p1[:], w1s[:, 0:128], d[:, 0:BLK], start=True, stop=False
                )
                nc.tensor.matmul(
                    p1[:], w1s[:, 128:256], d[:, 128 : 128 + BLK],
                    start=False, stop=True,
                )
                r1 = rpool.tile([128, BLK], dtm, tag="r1")
                nc.scalar.activation(
                    r1[:], p1[:], mybir.ActivationFunctionType.Relu
                )

                p2 = ps2.tile([64, BLK], f32, tag="p2")
                nc.tensor.matmul(p2[:], w2s[:], r1[:], start=True, stop=True)
                r2 = rpool.tile([64, BLK], dtm, tag="r2")
                nc.scalar.activation(
                    r2[:], p2[:], mybir.ActivationFunctionType.Relu
                )

                p3 = ps3.tile([128, 128], f32, tag="p3")
                for c in range(4):
                    nc.tensor.matmul(
                        p3[:, 32 * c : 32 * (c + 1)],
                        r2[:, 128 * c : 128 * (c + 1)],
                        w3s[:],
                        start=True,
                        stop=True,
                    )
                nc.vector.tensor_reduce(
                    isb[:, blk * 4 : (blk + 1) * 4],
                    p3[:].rearrange("p (c k) -> p c k", k=32),
                    axis=mybir.AxisListType.X,
                    op=mybir.AluOpType.max,
                )

            nc.sync.dma_start(iout[:], isb[:])

    nc.compile()
    return nc


def _get_compiled():
    global _compiled
    if _compiled is None:
        _compiled = _build()
    return _compiled


def _run_device(x, W1, W2, W3, trace=False):
    """Shard across 8 cores, run, return full pre-activation max array [N]."""
    from concourse.bass_utils import run_bass_kernel_spmd

    nc = _get_compiled()

    if USE_BF16:
        import ml_dtypes

        dt_np = ml_dtypes.bfloat16
    else:
        dt_np = np.float32

    x = np.ascontiguousarray(np.asarray(x, np.float32))
    xpad = np.zeros((NCORES - 1) * NPC + XSH, dt_np)
    xpad[:T] = x.astype(dt_np)
    w1 = np.ascontiguousarray(
        np.concatenate([W1.T[:128], W1.T[128:]], axis=1).astype(dt_np)
    )  # [128, 256]: [:, :128] = taps 0-127, [:, 128:] = taps 128-255
    w2 = np.ascontiguousarray(W2.T.astype(dt_np))  # [128, 64]
    w3 = np.ascontiguousarray(W3.T.astype(dt_np))  # [64, 32]

    in_maps = [
        {
            "xs": np.ascontiguousarray(xpad[i * NPC : i * NPC + XSH]),
            "w1t": w1,
            "w2t": w2,
            "w3t": w3,
        }
        for i in range(NCORES)
    ]
    res = run_bass_kernel_spmd(
        nc, in_maps, core_ids=list(range(NCORES)), trace=trace
    )

    maxpre = np.empty(N, np.float32)
    for i in range(NCORES):
        arr = res.results[i]["iout"]  # [128, NBLK*4]; arr[p, 4*blk + c]
        # local window n = 512*blk + 128*c + p
        loc = arr.reshape(128, NBLK, 4).transpose(1, 2, 0).reshape(-1)
        s = i * NPC
        cnt = min(NPC, N - s)
        maxpre[s : s + cnt] = loc[:cnt]
    return maxpre, res


def _host_finish(maxpre, x, W1, W2, W3):
    """Replicate the reference's LIF chain + argmin + winner (f32, host)."""
    import jax
    import jax.numpy as jnp

    I = jnp.maximum(jnp.asarray(maxpre), 0.0) * 2.0
    safe = jnp.where(I > 1.0, 1.0 - 1.0 / jnp.maximum(I, 1.0 + 1e-12), 0.5)
    n = jnp.maximum(
        jnp.ceil(jnp.log(safe) / jnp.log(jnp.float32(DECAY))), 1.0
    )
    spikes = (I > 1.0) & (n <= MAX_STEPS)
    latency = jnp.where(spikes, n * jnp.float32(DT), jnp.inf)
    abs_times = jnp.arange(N, dtype=jnp.float32) + latency
    best = jnp.argmin(abs_times)

    b = int(best)
    xb = jnp.asarray(np.asarray(x, np.float32)[b : b + W_WIN])
    h1 = jax.nn.relu(xb @ jnp.asarray(W1).T)
    h2 = jax.nn.relu(h1 @ jnp.asarray(W2).T)
    h3 = jax.nn.relu(h2 @ jnp.asarray(W3).T)
    winner = jnp.argmax(h3)

    # recompute the reported latency/abs_time from the f32 window so the
    # scalars match the reference's f32 chain bit-for-bit
    Ib = jnp.max(h3) * 2.0
    safeb = jnp.where(Ib > 1.0, 1.0 - 1.0 / jnp.maximum(Ib, 1.0 + 1e-12), 0.5)
    nb = jnp.maximum(
        jnp.ceil(jnp.log(safeb) / jnp.log(jnp.float32(DECAY))), 1.0
    )
    spikeb = (Ib > 1.0) & (nb <= MAX_STEPS)
    latb = jnp.where(spikeb, nb * jnp.float32(DT), jnp.inf)
    absb = jnp.float32(b) + latb

    return (
        np.asarray(best),
        np.asarray(winner),
        np.asarray(latb),
        np.asarray(absb),
    )


def kernel(x, W1, W2, W3):
    maxpre, _ = _run_device(x, W1, W2, W3)
    return _host_finish(maxpre, x, W1, W2, W3)


# revision 10
# speedup vs baseline: 1.3660x; 1.3660x over previous
"""Trainium2 Bass kernel for nn_DeepDendriticEncoder.

Computes, for every sliding window n of length 256 over x[0:500000]:
    h1 = relu(X @ W1.T); h2 = relu(h1 @ W2.T); h3 = relu(h2 @ W3.T)
    I[n] = 2 * max_k h3[n, k]
on 8 NeuronCores (window axis sharded, W-1 halo on x), then finishes the
tiny LIF latency / argmin chain on host in f32.

Device strategy per core (Hankel windows never materialized in DRAM):
  - per super-iteration of 4 blocks x 512 windows, one DMA brings a
    "diagonal" tile D[i, c] = x[base + i + c] (128 x 2176, overlapping
    strided read straight from HBM; big rows amortize descriptor cost)
  - conv-as-matmul: h1 = W1a.T @ D[:, b:b+512] + W1b.T @ D[:, b+128:b+640]
    accumulated in PSUM (contraction = tap index, 2 x 128); the weight
    loads amortize over the 4 blocks of a super-iteration
  - layer 2 packs two blocks per PSUM tile via column tiling
    (tile_position=(0,0)/(0,64)) so the two matmuls run concurrently on
    array column halves and relu processes 128 full partitions
  - layer 3 swaps operands (relu(h2) chunks stationary, W3.T moving) so
    h3 lands [window, k3]; block pairs run concurrently on array row
    halves (tile_position=(0,0)/(64,0)); the max over k3 is then a
    free-axis DVE reduce - no cross-partition reduction anywhere
  - per-core I values accumulate in SBUF, one DMA out at the end.

Matmul inputs run in bf16 (fp32 PSUM accumulation). The downstream
consumers are cliff functions with enormous margins for this problem
family (spike threshold I>1, integer step counts, argmax gaps), so
bf16-level error (~1e-2 relative) is far below every decision margin;
the reported winner/latency values are recomputed on host in f32.
"""

import sys

for _p in ("/opt/trn_rl_repo",):
    if _p not in sys.path:
        sys.path.insert(0, _p)

import numpy as np

# ---- problem constants (match reference.py; hardcoded by contract) ----
T = 500000
W_WIN = 256
K = 128
DT = 0.01
TAU = 0.05
DECAY = 1.0 - DT / TAU  # 0.8
MAX_STEPS = 200000
N = T - W_WIN + 1  # 499745

NCORES = 8
NPC = (N + NCORES - 1) // NCORES  # 62469 windows per core (last core fewer)
BLK = 512
SUPER = 4  # blocks per super-iteration (one diag DMA, 8 PSUM banks)
NSUP = (NPC + SUPER * BLK - 1) // (SUPER * BLK)  # 31
NBLK = NSUP * SUPER  # 124
CAP = NBLK * BLK  # 63488 windows computed per core (incl. padding)
XSH = CAP + 2 * K  # 63744 x-shard length

_compiled = None


def _build():
    """Build + compile the SPMD Bass program once per process."""
    import concourse.bass as bass
    import concourse.tile as tile
    from concourse import bacc, mybir

    f32 = mybir.dt.float32
    bf16 = mybir.dt.bfloat16
    RELU = mybir.ActivationFunctionType.Relu
    nc = bacc.Bacc("TRN2", target_bir_lowering=False)

    xs = nc.dram_tensor("xs", [XSH], bf16, kind="ExternalInput")
    w1t = nc.dram_tensor("w1t", [128, 256], bf16, kind="ExternalInput")
    w2t = nc.dram_tensor("w2t", [128, 64], bf16, kind="ExternalInput")
    # W3.T duplicated on partition halves so row-tiled layer-3 matmuls can
    # read it from partitions 0-63 and 64-127
    w3t = nc.dram_tensor("w3t", [128, 32], bf16, kind="ExternalInput")
    iout = nc.dram_tensor("iout", [128, NBLK * 4], f32, kind="ExternalOutput")

    DW = SUPER * BLK + 128  # 2176: diag tile width per super-iteration

    with tile.TileContext(nc) as tc:
        with (
            tc.tile_pool(name="const", bufs=1) as cpool,
            tc.tile_pool(name="diag", bufs=2) as dpool,
            tc.tile_pool(name="acts", bufs=2) as rpool,
            tc.tile_pool(name="iacc", bufs=1) as ipool,
            tc.tile_pool(name="psA", bufs=1, space="PSUM") as psA,
            tc.tile_pool(name="psB", bufs=1, space="PSUM") as psB,
            tc.tile_pool(name="psC", bufs=1, space="PSUM") as psC,
        ):
            w1s = cpool.tile([128, 256], bf16)
            nc.sync.dma_start(w1s[:], w1t[:])
            w2s = cpool.tile([128, 64], bf16)
            nc.sync.dma_start(w2s[:], w2t[:])
            w3s = cpool.tile([128, 32], bf16)
            nc.sync.dma_start(w3s[:], w3t[:])
            isb = ipool.tile([128, NBLK * 4], f32)

            for s in range(NSUP):
                base = s * SUPER * BLK
                d = dpool.tile([128, DW], bf16, tag="d")
                nc.sync.dma_start(d[:], bass.AP(xs, base, [[1, 128], [1, DW]]))

                # layer 1: 4 blocks, weight halves loaded once each
                p1 = [
                    psA.tile([128, BLK], f32, name=f"p1_{k}", tag=f"p1_{k}")
                    for k in range(SUPER)
                ]
                for k in range(SUPER):
                    nc.tensor.matmul(
                        p1[k][:], w1s[:, 0:128], d[:, BLK * k : BLK * k + BLK],
                        start=True, stop=False,
                    )
                for k in range(SUPER):
                    nc.tensor.matmul(
                        p1[k][:], w1s[:, 128:256],
                        d[:, BLK * k + 128 : BLK * k + 128 + BLK],
                        start=False, stop=True,
                    )
                r1 = []
                for k in range(SUPER):
                    t = rpool.tile([128, BLK], bf16, name=f"r1_{k}", tag=f"r1_{k}")
                    nc.scalar.activation(t[:], p1[k][:], RELU)
                    r1.append(t)

                # layer 2: block pairs packed on array column halves
                r2 = []
                for j in range(SUPER // 2):
                    p2 = psB.tile([128, BLK], f32, name=f"p2_{j}", tag=f"p2_{j}")
                    nc.tensor.matmul(
                        p2[0:64, :], w2s[:], r1[2 * j][:],
                        start=True, stop=True, tile_position=(0, 0),
                    )
                    nc.tensor.matmul(
                        p2[64:128, :], w2s[:], r1[2 * j + 1][:],
                        start=True, stop=True, tile_position=(0, 64),
                    )
                    t = rpool.tile([128, BLK], bf16, name=f"r2_{j}", tag=f"r2_{j}")
                    nc.vector.tensor_relu(t[:], p2[:])
                    r2.append(t)

                # layer 3: stationary = relu(h2) chunks, moving = W3.T;
                # block pairs run concurrently on array row halves, each
                # row group draining into its own PSUM bank
                p3a = psC.tile([128, 256], f32, name="p3a", tag="p3a")
                p3b = psC.tile([128, 256], f32, name="p3b", tag="p3b")
                for j in range(SUPER // 2):
                    for c in range(4):
                        nc.tensor.matmul(
                            p3a[:, 128 * j + 32 * c : 128 * j + 32 * c + 32],
                            r2[j][0:64, 128 * c : 128 * (c + 1)],
                            w3s[0:64, :],
                            start=True, stop=True, tile_position=(0, 0),
                        )
                        nc.tensor.matmul(
                            p3b[:, 128 * j + 32 * c : 128 * j + 32 * c + 32],
                            r2[j][64:128, 128 * c : 128 * (c + 1)],
                            w3s[64:128, :],
                            start=True, stop=True, tile_position=(64, 0),
                        )
                # p3a col 128j+32c <-> block 4s+2j,   window 512 blk + 128 c + p
                # p3b col 128j+32c <-> block 4s+2j+1, window 512 blk + 128 c + p
                nc.vector.tensor_reduce(
                    isb[:, 16 * s : 16 * s + 8],
                    p3a[:].rearrange("p (g k) -> p g k", k=32),
                    axis=mybir.AxisListType.X,
                    op=mybir.AluOpType.max,
                )
                nc.vector.tensor_reduce(
                    isb[:, 16 * s + 8 : 16 * s + 16],
                    p3b[:].rearrange("p (g k) -> p g k", k=32),
                    axis=mybir.AxisListType.X,
                    op=mybir.AluOpType.max,
                )

            nc.sync.dma_start(iout[:], isb[:])

    nc.compile()
    return nc


def _get_compiled():
    global _compiled
    if _compiled is None:
        _compiled = _build()
    return _compiled


def _run_device(x, W1, W2, W3, trace=False):
    """Shard across 8 cores, run, return full pre-activation max array [N]."""
    import ml_dtypes
    from concourse.bass_utils import run_bass_kernel_spmd

    nc = _get_compiled()
    bf = ml_dtypes.bfloat16

    x = np.ascontiguousarray(np.asarray(x, np.float32))
    xpad = np.zeros((NCORES - 1) * NPC + XSH, bf)
    xpad[:T] = x.astype(bf)
    w1 = np.ascontiguousarray(
        np.concatenate([W1.T[:128], W1.T[128:]], axis=1).astype(bf)
    )  # [128, 256]: [:, :128] = taps 0-127, [:, 128:] = taps 128-255
    w2 = np.ascontiguousarray(W2.T.astype(bf))  # [128, 64]
    w3 = np.ascontiguousarray(
        np.concatenate([W3.T, W3.T], axis=0).astype(bf)
    )  # [128, 32] = W3.T stacked twice

    in_maps = [
        {
            "xs": np.ascontiguousarray(xpad[i * NPC : i * NPC + XSH]),
            "w1t": w1,
            "w2t": w2,
            "w3t": w3,
        }
        for i in range(NCORES)
    ]
    res = run_bass_kernel_spmd(
        nc, in_maps, core_ids=list(range(NCORES)), trace=trace
    )

    maxpre = np.empty(N, np.float32)
    for i in range(NCORES):
        arr = res.results[i]["iout"]  # [128, NBLK*4]
        # col = 16 s + 8 par + 4 j + c maps to block 4s + 2j + par;
        # window n = 512 blk + 128 c + p = 2048 s + 1024 j + 512 par + 128 c + p
        loc = (
            arr.reshape(128, NSUP, 2, 2, 4)  # p, s, par, j, c
            .transpose(1, 3, 2, 4, 0)  # s, j, par, c, p
            .reshape(-1)
        )
        s = i * NPC
        cnt = min(NPC, N - s)
        maxpre[s : s + cnt] = loc[:cnt]
    return maxpre, res


def _host_finish(maxpre, x, W1, W2, W3):
    """Replicate the reference's LIF chain + argmin + winner (f32, host)."""
    f32 = np.float32
    I = (np.maximum(maxpre, 0) * f32(2.0)).astype(f32)
    safe = np.where(
        I > 1.0, f32(1.0) - f32(1.0) / np.maximum(I, f32(1.0 + 1e-12)), f32(0.5)
    ).astype(f32)
    n = np.maximum(np.ceil(np.log(safe) / np.log(f32(DECAY))), f32(1.0)).astype(f32)
    spikes = (I > 1.0) & (n <= MAX_STEPS)
    latency = np.where(spikes, n * f32(DT), f32(np.inf)).astype(f32)
    abs_times = (np.arange(N, dtype=f32) + latency).astype(f32)
    best = int(np.argmin(abs_times))

    # recompute the reported values from the f32 window (matches the
    # reference's f32 chain; device bf16 only picks the argmin window)
    xw = np.asarray(x, f32)[best : best + W_WIN]
    W1f = np.asarray(W1, f32)
    W2f = np.asarray(W2, f32)
    W3f = np.asarray(W3, f32)
    h1 = np.maximum(W1f @ xw, 0)
    h2 = np.maximum(W2f @ h1, 0)
    h3 = np.maximum(W3f @ h2, 0)
    winner = int(np.argmax(h3))

    Ib = f32(h3.max() * f32(2.0))
    safeb = (
        f32(1.0) - f32(1.0) / max(Ib, f32(1.0 + 1e-12)) if Ib > 1.0 else f32(0.5)
    )
    nb = f32(max(np.ceil(np.log(f32(safeb)) / np.log(f32(DECAY))), 1.0))
    spikeb = (Ib > 1.0) and (nb <= MAX_STEPS)
    latb = f32(nb * f32(DT)) if spikeb else f32(np.inf)
    absb = f32(f32(best) + latb)

    return (
        np.int32(best),
        np.int32(winner),
        f32(latb),
        f32(absb),
    )


def kernel(x, W1, W2, W3):
    maxpre, _ = _run_device(x, W1, W2, W3)
    return _host_finish(maxpre, x, W1, W2, W3)


# revision 17
# speedup vs baseline: 1.4432x; 1.0565x over previous
"""Trainium2 Bass kernel for nn_DeepDendriticEncoder.

Computes, for every sliding window n of length 256 over x[0:500000]:
    h1 = relu(X @ W1.T); h2 = relu(h1 @ W2.T); h3 = relu(h2 @ W3.T)
    I[n] = 2 * max_k h3[n, k]
on 8 NeuronCores (window axis sharded, W-1 halo on x), then finishes the
tiny LIF latency / argmin chain on host in f32.

Device strategy per core (Hankel windows never materialized in DRAM):
  - per super-iteration of 4 blocks x 512 windows, one DMA brings a
    "diagonal" tile D[i, c] = x[base + i + c] (128 x 2176, overlapping
    strided read straight from HBM; big rows amortize descriptor cost)
  - conv-as-matmul: h1 = W1a.T @ D[:, b:b+512] + W1b.T @ D[:, b+128:b+640]
    accumulated in PSUM (contraction = tap index, 2 x 128); the weight
    loads amortize over the 4 blocks of a super-iteration
  - layer 2 packs two blocks per PSUM tile via column tiling
    (tile_position=(0,0)/(0,64)) so the two matmuls run concurrently on
    array column halves and relu processes 128 full partitions
  - layer 3 swaps operands (relu(h2) chunks stationary, W3.T moving) so
    h3 lands [window, k3]; block pairs run concurrently on array row
    halves (tile_position=(0,0)/(64,0)); the max over k3 is then a
    free-axis DVE reduce - no cross-partition reduction anywhere
  - per-core I values accumulate in SBUF, one DMA out at the end.

Matmul inputs run in bf16 (fp32 PSUM accumulation). The downstream
consumers are cliff functions with enormous margins for this problem
family (spike threshold I>1, integer step counts, argmax gaps), so
bf16-level error (~1e-2 relative) is far below every decision margin;
the reported winner/latency values are recomputed on host in f32.
"""

import sys

for _p in ("/opt/trn_rl_repo",):
    if _p not in sys.path:
        sys.path.insert(0, _p)

import numpy as np

# ---- problem constants (match reference.py; hardcoded by contract) ----
T = 500000
W_WIN = 256
K = 128
DT = 0.01
TAU = 0.05
DECAY = 1.0 - DT / TAU  # 0.8
MAX_STEPS = 200000
N = T - W_WIN + 1  # 499745

NCORES = 8
NPC = (N + NCORES - 1) // NCORES  # 62469 windows per core (last core fewer)
BLK = 512
SUPER = 4  # blocks per super-iteration (one diag DMA, 8 PSUM banks)
NSUP = (NPC + SUPER * BLK - 1) // (SUPER * BLK)  # 31
NBLK = NSUP * SUPER  # 124
CAP = NBLK * BLK  # 63488 windows computed per core (incl. padding)
XSH = CAP + 2 * K  # 63744 x-shard length

_compiled = None


def _build():
    """Build + compile the SPMD Bass program once per process."""
    import concourse.bass as bass
    import concourse.tile as tile
    from concourse import bacc, mybir

    f32 = mybir.dt.float32
    # fp8 data path, fp32 PSUM accumulation. x and W1 fit comfortably in
    # e3m4's +-15.5 range (|x| < 5, |W1| < 1.5); relu(h1)/relu(h2) can
    # exceed 15.5, so layers 2/3 use e4m3 (+-448).
    dt1 = mybir.dt.float8e3
    dt2 = mybir.dt.float8e4
    RELU = mybir.ActivationFunctionType.Relu
    nc = bacc.Bacc("TRN2", target_bir_lowering=False)

    xs = nc.dram_tensor("xs", [XSH], dt1, kind="ExternalInput")
    w1t = nc.dram_tensor("w1t", [128, 256], dt1, kind="ExternalInput")
    w2t = nc.dram_tensor("w2t", [128, 64], dt2, kind="ExternalInput")
    # W3.T duplicated on partition halves so row-tiled layer-3 matmuls can
    # read it from partitions 0-63 and 64-127
    w3t = nc.dram_tensor("w3t", [128, 32], dt2, kind="ExternalInput")
    iout = nc.dram_tensor("iout", [128, NBLK * 4], f32, kind="ExternalOutput")

    DW = SUPER * BLK + 128  # 2176: diag tile width per super-iteration

    with tile.TileContext(nc) as tc:
        with (
            tc.tile_pool(name="const", bufs=1) as cpool,
            tc.tile_pool(name="diag", bufs=2) as dpool,
            tc.tile_pool(name="acts", bufs=2) as rpool,
            tc.tile_pool(name="iacc", bufs=1) as ipool,
            tc.tile_pool(name="psA", bufs=1, space="PSUM") as psA,
            tc.tile_pool(name="psB", bufs=1, space="PSUM") as psB,
            tc.tile_pool(name="psC", bufs=1, space="PSUM") as psC,
        ):
            w1s = cpool.tile([128, 256], dt1)
            nc.sync.dma_start(w1s[:], w1t[:])
            w2s = cpool.tile([128, 64], dt2)
            nc.sync.dma_start(w2s[:], w2t[:])
            w3s = cpool.tile([128, 32], dt2)
            nc.sync.dma_start(w3s[:], w3t[:])
            isb = ipool.tile([128, NBLK * 4], f32)

            for s in range(NSUP):
                base = s * SUPER * BLK
                d = dpool.tile([128, DW], dt1, tag="d")
                nc.sync.dma_start(d[:], bass.AP(xs, base, [[1, 128], [1, DW]]))

                # layer 1: 4 blocks, weight halves loaded once each
                p1 = [
                    psA.tile([128, BLK], f32, name=f"p1_{k}", tag=f"p1_{k}")
                    for k in range(SUPER)
                ]
                for k in range(SUPER):
                    nc.tensor.matmul(
                        p1[k][:], w1s[:, 0:128], d[:, BLK * k : BLK * k + BLK],
                        start=True, stop=False,
                    )
                for k in range(SUPER):
                    nc.tensor.matmul(
                        p1[k][:], w1s[:, 128:256],
                        d[:, BLK * k + 128 : BLK * k + 128 + BLK],
                        start=False, stop=True,
                    )
                r1 = []
                for k in range(SUPER):
                    t = rpool.tile([128, BLK], dt2, name=f"r1_{k}", tag=f"r1_{k}")
                    nc.scalar.activation(t[:], p1[k][:], RELU)
                    r1.append(t)

                # layer 2: block pairs packed on array column halves
                r2 = []
                for j in range(SUPER // 2):
                    p2 = psB.tile([128, BLK], f32, name=f"p2_{j}", tag=f"p2_{j}")
                    nc.tensor.matmul(
                        p2[0:64, :], w2s[:], r1[2 * j][:],
                        start=True, stop=True, tile_position=(0, 0),
                    )
                    nc.tensor.matmul(
                        p2[64:128, :], w2s[:], r1[2 * j + 1][:],
                        start=True, stop=True, tile_position=(0, 64),
                    )
                    t = rpool.tile([128, BLK], dt2, name=f"r2_{j}", tag=f"r2_{j}")
                    nc.vector.tensor_relu(t[:], p2[:])
                    r2.append(t)

                # layer 3: stationary = relu(h2) chunks, moving = W3.T;
                # block pairs run concurrently on array row halves, each
                # row group draining into its own PSUM bank
                p3a = psC.tile([128, 256], f32, name="p3a", tag="p3a")
                p3b = psC.tile([128, 256], f32, name="p3b", tag="p3b")
                for j in range(SUPER // 2):
                    for c in range(4):
                        nc.tensor.matmul(
                            p3a[:, 128 * j + 32 * c : 128 * j + 32 * c + 32],
                            r2[j][0:64, 128 * c : 128 * (c + 1)],
                            w3s[0:64, :],
                            start=True, stop=True, tile_position=(0, 0),
                        )
                        nc.tensor.matmul(
                            p3b[:, 128 * j + 32 * c : 128 * j + 32 * c + 32],
                            r2[j][64:128, 128 * c : 128 * (c + 1)],
                            w3s[64:128, :],
                            start=True, stop=True, tile_position=(64, 0),
                        )
                # p3a col 128j+32c <-> block 4s+2j,   window 512 blk + 128 c + p
                # p3b col 128j+32c <-> block 4s+2j+1, window 512 blk + 128 c + p
                nc.vector.tensor_reduce(
                    isb[:, 16 * s : 16 * s + 8],
                    p3a[:].rearrange("p (g k) -> p g k", k=32),
                    axis=mybir.AxisListType.X,
                    op=mybir.AluOpType.max,
                )
                nc.vector.tensor_reduce(
                    isb[:, 16 * s + 8 : 16 * s + 16],
                    p3b[:].rearrange("p (g k) -> p g k", k=32),
                    axis=mybir.AxisListType.X,
                    op=mybir.AluOpType.max,
                )

            nc.sync.dma_start(iout[:], isb[:])

    nc.compile()
    return nc


def _get_compiled():
    global _compiled
    if _compiled is None:
        _compiled = _build()
    return _compiled


def _run_device(x, W1, W2, W3, trace=False):
    """Shard across 8 cores, run, return full pre-activation max array [N]."""
    import ml_dtypes
    from concourse.bass_utils import run_bass_kernel_spmd

    nc = _get_compiled()
    f8a = ml_dtypes.float8_e3m4
    f8b = ml_dtypes.float8_e4m3

    x = np.ascontiguousarray(np.asarray(x, np.float32))
    xpad = np.zeros((NCORES - 1) * NPC + XSH, f8a)
    xpad[:T] = np.clip(x, -15.5, 15.5).astype(f8a)
    w1 = np.ascontiguousarray(
        np.clip(np.concatenate([W1.T[:128], W1.T[128:]], axis=1), -15.5, 15.5)
        .astype(f8a)
    )  # [128, 256]: [:, :128] = taps 0-127, [:, 128:] = taps 128-255
    w2 = np.ascontiguousarray(W2.T.astype(f8b))  # [128, 64]
    w3 = np.ascontiguousarray(
        np.concatenate([W3.T, W3.T], axis=0).astype(f8b)
    )  # [128, 32] = W3.T stacked twice

    in_maps = [
        {
            "xs": np.ascontiguousarray(xpad[i * NPC : i * NPC + XSH]),
            "w1t": w1,
            "w2t": w2,
            "w3t": w3,
        }
        for i in range(NCORES)
    ]
    res = run_bass_kernel_spmd(
        nc, in_maps, core_ids=list(range(NCORES)), trace=trace
    )

    maxpre = np.empty(N, np.float32)
    for i in range(NCORES):
        arr = res.results[i]["iout"]  # [128, NBLK*4]
        # col = 16 s + 8 par + 4 j + c maps to block 4s + 2j + par;
        # window n = 512 blk + 128 c + p = 2048 s + 1024 j + 512 par + 128 c + p
        loc = (
            arr.reshape(128, NSUP, 2, 2, 4)  # p, s, par, j, c
            .transpose(1, 3, 2, 4, 0)  # s, j, par, c, p
            .reshape(-1)
        )
        s = i * NPC
        cnt = min(NPC, N - s)
        maxpre[s : s + cnt] = loc[:cnt]
    return maxpre, res


def _host_finish(maxpre, x, W1, W2, W3):
    """Replicate the reference's LIF chain + argmin + winner (f32, host)."""
    f32 = np.float32
    I = (np.maximum(maxpre, 0) * f32(2.0)).astype(f32)
    safe = np.where(
        I > 1.0, f32(1.0) - f32(1.0) / np.maximum(I, f32(1.0 + 1e-12)), f32(0.5)
    ).astype(f32)
    n = np.maximum(np.ceil(np.log(safe) / np.log(f32(DECAY))), f32(1.0)).astype(f32)
    spikes = (I > 1.0) & (n <= MAX_STEPS)
    latency = np.where(spikes, n * f32(DT), f32(np.inf)).astype(f32)
    abs_times = (np.arange(N, dtype=f32) + latency).astype(f32)
    best = int(np.argmin(abs_times))

    # recompute the reported values from the f32 window (matches the
    # reference's f32 chain; device bf16 only picks the argmin window)
    xw = np.asarray(x, f32)[best : best + W_WIN]
    W1f = np.asarray(W1, f32)
    W2f = np.asarray(W2, f32)
    W3f = np.asarray(W3, f32)
    h1 = np.maximum(W1f @ xw, 0)
    h2 = np.maximum(W2f @ h1, 0)
    h3 = np.maximum(W3f @ h2, 0)
    winner = int(np.argmax(h3))

    Ib = f32(h3.max() * f32(2.0))
    safeb = (
        f32(1.0) - f32(1.0) / max(Ib, f32(1.0 + 1e-12)) if Ib > 1.0 else f32(0.5)
    )
    nb = f32(max(np.ceil(np.log(f32(safeb)) / np.log(f32(DECAY))), 1.0))
    spikeb = (Ib > 1.0) and (nb <= MAX_STEPS)
    latb = f32(nb * f32(DT)) if spikeb else f32(np.inf)
    absb = f32(f32(best) + latb)

    return (
        np.int32(best),
        np.int32(winner),
        f32(latb),
        f32(absb),
    )


def kernel(x, W1, W2, W3):
    maxpre, _ = _run_device(x, W1, W2, W3)
    return _host_finish(maxpre, x, W1, W2, W3)


# revision 26
# speedup vs baseline: 1.7101x; 1.1849x over previous
"""Trainium2 Bass kernel for nn_DeepDendriticEncoder.

Computes, for every sliding window n of length 256 over x[0:500000]:
    h1 = relu(X @ W1.T); h2 = relu(h1 @ W2.T); h3 = relu(h2 @ W3.T)
    I[n] = 2 * max_k h3[n, k]
on 8 NeuronCores (window axis sharded, W-1 halo on x), then finishes the
tiny LIF latency / argmin chain on host in f32.

Device strategy per core (Hankel windows never materialized in DRAM):
  - per super-iteration of 4 blocks x 512 windows, one DMA brings a
    "diagonal" tile D[i, c] = x[base + i + c] (128 x 2176, overlapping
    strided read straight from HBM; big rows amortize descriptor cost)
  - conv-as-matmul: h1 = W1a.T @ D[:, b:b+512] + W1b.T @ D[:, b+128:b+640]
    accumulated in PSUM (contraction = tap index, 2 x 128); the weight
    loads amortize over the 4 blocks of a super-iteration
  - layer 2 packs two blocks per PSUM tile via column tiling
    (tile_position=(0,0)/(0,64)) so the two matmuls run concurrently on
    array column halves and relu processes 128 full partitions
  - layer 3 swaps operands (relu(h2) chunks stationary, W3.T moving) so
    h3 lands [window, k3]; block pairs run concurrently on array row
    halves (tile_position=(0,0)/(64,0)); the max over k3 is then a
    free-axis DVE reduce - no cross-partition reduction anywhere
  - per-core I values accumulate in SBUF, one DMA out at the end.

Matmul inputs run in bf16 (fp32 PSUM accumulation). The downstream
consumers are cliff functions with enormous margins for this problem
family (spike threshold I>1, integer step counts, argmax gaps), so
bf16-level error (~1e-2 relative) is far below every decision margin;
the reported winner/latency values are recomputed on host in f32.
"""

import sys

for _p in ("/opt/trn_rl_repo",):
    if _p not in sys.path:
        sys.path.insert(0, _p)

import numpy as np

# ---- problem constants (match reference.py; hardcoded by contract) ----
T = 500000
W_WIN = 256
K = 128
DT = 0.01
TAU = 0.05
DECAY = 1.0 - DT / TAU  # 0.8
MAX_STEPS = 200000
N = T - W_WIN + 1  # 499745

NCORES = 8
NPC = (N + NCORES - 1) // NCORES  # 62469 windows per core (last core fewer)
BLK = 512
SUPER = 4  # blocks per super-iteration (one diag DMA, 8 PSUM banks)
NSUP = (NPC + SUPER * BLK - 1) // (SUPER * BLK)  # 31
NBLK = NSUP * SUPER  # 124
CAP = NBLK * BLK  # 63488 windows computed per core (incl. padding)
XSH = CAP + 2 * K  # 63744 x-shard length

_compiled = None


def _build():
    """Build + compile the SPMD Bass program once per process."""
    import concourse.bass as bass
    import concourse.tile as tile
    from concourse import bacc, mybir

    f32 = mybir.dt.float32
    # fp8 e4m3 data path, fp32 PSUM accumulation. Layer 1 runs in
    # DoubleRow perf mode (2 fp8 MACs/cell/cycle, contraction 256 in one
    # matmul); e4m3's +-448 range covers every operand comfortably.
    dt1 = mybir.dt.float8e4
    dt2 = mybir.dt.float8e4
    RELU = mybir.ActivationFunctionType.Relu
    nc = bacc.Bacc("TRN2", target_bir_lowering=False)

    xs = nc.dram_tensor("xs", [XSH], dt1, kind="ExternalInput")
    w1t = nc.dram_tensor("w1t", [128, 256], dt1, kind="ExternalInput")
    w2t = nc.dram_tensor("w2t", [128, 64], dt2, kind="ExternalInput")
    # W3.T duplicated on partition halves so row-tiled layer-3 matmuls can
    # read it from partitions 0-63 and 64-127
    w3t = nc.dram_tensor("w3t", [128, 32], dt2, kind="ExternalInput")
    iout = nc.dram_tensor("iout", [128, NBLK * 4], f32, kind="ExternalOutput")

    DW = SUPER * BLK + 128  # 2176: diag tile width per super-iteration

    with tile.TileContext(nc) as tc:
        with (
            tc.tile_pool(name="const", bufs=1) as cpool,
            tc.tile_pool(name="diag", bufs=2) as dpool,
            tc.tile_pool(name="acts", bufs=2) as rpool,
            tc.tile_pool(name="iacc", bufs=1) as ipool,
            tc.tile_pool(name="psA", bufs=1, space="PSUM") as psA,
            tc.tile_pool(name="psB", bufs=1, space="PSUM") as psB,
            tc.tile_pool(name="psC", bufs=1, space="PSUM") as psC,
        ):
            w1s = cpool.tile([128, 256], dt1)
            nc.sync.dma_start(w1s[:], w1t[:])
            w2s = cpool.tile([128, 64], dt2)
            nc.sync.dma_start(w2s[:], w2t[:])
            w3s = cpool.tile([128, 32], dt2)
            nc.sync.dma_start(w3s[:], w3t[:])
            isb = ipool.tile([128, NBLK * 4], f32)

            SW = SUPER * BLK  # 2048 windows per super-iteration
            for ds in range(0, NSUP, 2):
                span = min(2, NSUP - ds)
                dw = span * SW + 128
                d = dpool.tile([128, 2 * SW + 128], dt1, tag="d")
                nc.sync.dma_start(
                    d[:, :dw], bass.AP(xs, ds * SW, [[1, 128], [1, dw]])
                )
                # DoubleRow APs: contraction = (partition i, ktile q) over
                # taps 128 q + i; free dims [q, elem] with q-step 128
                w1dr = bass.AP(
                    w1s[:].tensor, w1s[:].offset,
                    [list(w1s[:].ap[0]), [128, 2], [1, 128]],
                )
                for s in range(ds, ds + span):
                    off = (s - ds) * SW
                    # layer 1: one DoubleRow matmul per 512-window block
                    p1 = [
                        psA.tile([128, BLK], f32, name=f"p1_{k}", tag=f"p1_{k}")
                        for k in range(SUPER)
                    ]
                    for k in range(SUPER):
                        dsl = d[:, off + BLK * k : off + BLK * k + BLK + 128]
                        ddr = bass.AP(
                            dsl.tensor, dsl.offset,
                            [list(dsl.ap[0]), [128, 2], [1, BLK]],
                        )
                        nc.tensor.matmul(
                            p1[k][:], w1dr, ddr,
                            start=True, stop=True,
                            perf_mode=mybir.MatmulPerfMode.DoubleRow,
                        )
                    r1 = []
                    for k in range(SUPER):
                        t = rpool.tile(
                            [128, BLK], dt2, name=f"r1_{k}", tag=f"r1_{k}"
                        )
                        nc.scalar.activation(t[:], p1[k][:], RELU)
                        r1.append(t)

                    # layer 2: block pairs packed on array column halves
                    r2 = []
                    for j in range(SUPER // 2):
                        p2 = psB.tile(
                            [128, BLK], f32, name=f"p2_{j}", tag=f"p2_{j}"
                        )
                        nc.tensor.matmul(
                            p2[0:64, :], w2s[:], r1[2 * j][:],
                            start=True, stop=True, tile_position=(0, 0),
                        )
                        nc.tensor.matmul(
                            p2[64:128, :], w2s[:], r1[2 * j + 1][:],
                            start=True, stop=True, tile_position=(0, 64),
                        )
                        t = rpool.tile(
                            [128, BLK], dt2, name=f"r2_{j}", tag=f"r2_{j}"
                        )
                        nc.vector.tensor_relu(t[:], p2[:])
                        r2.append(t)

                    # layer 3: stationary = relu(h2) chunks, moving = W3.T;
                    # block pairs run concurrently on array row halves, each
                    # row group draining into its own PSUM bank
                    p3a = psC.tile([128, 256], f32, name="p3a", tag="p3a")
                    p3b = psC.tile([128, 256], f32, name="p3b", tag="p3b")
                    for j in range(SUPER // 2):
                        for c in range(4):
                            nc.tensor.matmul(
                                p3a[:, 128 * j + 32 * c : 128 * j + 32 * c + 32],
                                r2[j][0:64, 128 * c : 128 * (c + 1)],
                                w3s[0:64, :],
                                start=True, stop=True, tile_position=(0, 0),
                            )
                            nc.tensor.matmul(
                                p3b[:, 128 * j + 32 * c : 128 * j + 32 * c + 32],
                                r2[j][64:128, 128 * c : 128 * (c + 1)],
                                w3s[64:128, :],
                                start=True, stop=True, tile_position=(64, 0),
                            )
                    # p3a col 128j+32c <-> block 4s+2j,   window 512 blk + 128 c + p
                    # p3b col 128j+32c <-> block 4s+2j+1, window 512 blk + 128 c + p
                    nc.vector.tensor_reduce(
                        isb[:, 16 * s : 16 * s + 8],
                        p3a[:].rearrange("p (g k) -> p g k", k=32),
                        axis=mybir.AxisListType.X,
                        op=mybir.AluOpType.max,
                    )
                    nc.vector.tensor_reduce(
                        isb[:, 16 * s + 8 : 16 * s + 16],
                        p3b[:].rearrange("p (g k) -> p g k", k=32),
                        axis=mybir.AxisListType.X,
                        op=mybir.AluOpType.max,
                    )

            nc.sync.dma_start(iout[:], isb[:])

    nc.compile()
    return nc


def _get_compiled():
    global _compiled
    if _compiled is None:
        _compiled = _build()
    return _compiled


def _run_device(x, W1, W2, W3, trace=False):
    """Shard across 8 cores, run, return full pre-activation max array [N]."""
    import ml_dtypes
    from concourse.bass_utils import run_bass_kernel_spmd

    nc = _get_compiled()
    f8a = ml_dtypes.float8_e4m3
    f8b = ml_dtypes.float8_e4m3

    x = np.ascontiguousarray(np.asarray(x, np.float32))
    xpad = np.zeros((NCORES - 1) * NPC + XSH, f8a)
    xpad[:T] = np.clip(x, -448, 448).astype(f8a)
    w1 = np.ascontiguousarray(
        np.clip(np.concatenate([W1.T[:128], W1.T[128:]], axis=1), -448, 448)
        .astype(f8a)
    )  # [128, 256]: [:, :128] = taps 0-127, [:, 128:] = taps 128-255
    w2 = np.ascontiguousarray(W2.T.astype(f8b))  # [128, 64]
    w3 = np.ascontiguousarray(
        np.concatenate([W3.T, W3.T], axis=0).astype(f8b)
    )  # [128, 32] = W3.T stacked twice

    in_maps = [
        {
            "xs": np.ascontiguousarray(xpad[i * NPC : i * NPC + XSH]),
            "w1t": w1,
            "w2t": w2,
            "w3t": w3,
        }
        for i in range(NCORES)
    ]
    res = run_bass_kernel_spmd(
        nc, in_maps, core_ids=list(range(NCORES)), trace=trace
    )

    maxpre = np.empty(N, np.float32)
    for i in range(NCORES):
        arr = res.results[i]["iout"]  # [128, NBLK*4]
        # col = 16 s + 8 par + 4 j + c maps to block 4s + 2j + par;
        # window n = 512 blk + 128 c + p = 2048 s + 1024 j + 512 par + 128 c + p
        loc = (
            arr.reshape(128, NSUP, 2, 2, 4)  # p, s, par, j, c
            .transpose(1, 3, 2, 4, 0)  # s, j, par, c, p
            .reshape(-1)
        )
        s = i * NPC
        cnt = min(NPC, N - s)
        maxpre[s : s + cnt] = loc[:cnt]
    return maxpre, res


def _host_finish(maxpre, x, W1, W2, W3):
    """Replicate the reference's LIF chain + argmin + winner (f32, host)."""
    f32 = np.float32
    I = (np.maximum(maxpre, 0) * f32(2.0)).astype(f32)
    safe = np.where(
        I > 1.0, f32(1.0) - f32(1.0) / np.maximum(I, f32(1.0 + 1e-12)), f32(0.5)
    ).astype(f32)
    n = np.maximum(np.ceil(np.log(safe) / np.log(f32(DECAY))), f32(1.0)).astype(f32)
    spikes = (I > 1.0) & (n <= MAX_STEPS)
    latency = np.where(spikes, n * f32(DT), f32(np.inf)).astype(f32)
    abs_times = (np.arange(N, dtype=f32) + latency).astype(f32)
    best = int(np.argmin(abs_times))

    # recompute the reported values from the f32 window (matches the
    # reference's f32 chain; device bf16 only picks the argmin window)
    xw = np.asarray(x, f32)[best : best + W_WIN]
    W1f = np.asarray(W1, f32)
    W2f = np.asarray(W2, f32)
    W3f = np.asarray(W3, f32)
    h1 = np.maximum(W1f @ xw, 0)
    h2 = np.maximum(W2f @ h1, 0)
    h3 = np.maximum(W3f @ h2, 0)
    winner = int(np.argmax(h3))

    Ib = f32(h3.max() * f32(2.0))
    safeb = (
        f32(1.0) - f32(1.0) / max(Ib, f32(1.0 + 1e-12)) if Ib > 1.0 else f32(0.5)
    )
    nb = f32(max(np.ceil(np.log(f32(safeb)) / np.log(f32(DECAY))), 1.0))
    spikeb = (Ib > 1.0) and (nb <= MAX_STEPS)
    latb = f32(nb * f32(DT)) if spikeb else f32(np.inf)
    absb = f32(f32(best) + latb)

    return (
        np.int32(best),
        np.int32(winner),
        f32(latb),
        f32(absb),
    )


def kernel(x, W1, W2, W3):
    maxpre, _ = _run_device(x, W1, W2, W3)
    return _host_finish(maxpre, x, W1, W2, W3)


# revision 30
# speedup vs baseline: 2.0275x; 1.1856x over previous
"""Trainium2 Bass kernel for nn_DeepDendriticEncoder.

Computes, for every sliding window n of length 256 over x[0:500000]:
    h1 = relu(X @ W1.T); h2 = relu(h1 @ W2.T); h3 = relu(h2 @ W3.T)
    I[n] = 2 * max_k h3[n, k]
on 8 NeuronCores (window axis sharded, W-1 halo on x), then finishes the
tiny LIF latency / argmin chain on host in f32.

Device strategy per core (Hankel windows never materialized in DRAM):
  - per super-iteration of 4 blocks x 512 windows, one DMA brings a
    "diagonal" tile D[i, c] = x[base + i + c] (128 x 2176, overlapping
    strided read straight from HBM; big rows amortize descriptor cost)
  - conv-as-matmul: h1 = W1a.T @ D[:, b:b+512] + W1b.T @ D[:, b+128:b+640]
    accumulated in PSUM (contraction = tap index, 2 x 128); the weight
    loads amortize over the 4 blocks of a super-iteration
  - layer 2 packs two blocks per PSUM tile via column tiling
    (tile_position=(0,0)/(0,64)) so the two matmuls run concurrently on
    array column halves and relu processes 128 full partitions
  - layer 3 swaps operands (relu(h2) chunks stationary, W3.T moving) so
    h3 lands [window, k3]; block pairs run concurrently on array row
    halves (tile_position=(0,0)/(64,0)); the max over k3 is then a
    free-axis DVE reduce - no cross-partition reduction anywhere
  - per-core I values accumulate in SBUF, one DMA out at the end.

Matmul inputs run in bf16 (fp32 PSUM accumulation). The downstream
consumers are cliff functions with enormous margins for this problem
family (spike threshold I>1, integer step counts, argmax gaps), so
bf16-level error (~1e-2 relative) is far below every decision margin;
the reported winner/latency values are recomputed on host in f32.
"""

import sys

for _p in ("/opt/trn_rl_repo",):
    if _p not in sys.path:
        sys.path.insert(0, _p)

import numpy as np

# ---- problem constants (match reference.py; hardcoded by contract) ----
T = 500000
W_WIN = 256
K = 128
DT = 0.01
TAU = 0.05
DECAY = 1.0 - DT / TAU  # 0.8
MAX_STEPS = 200000
N = T - W_WIN + 1  # 499745

NCORES = 8
NPC = (N + NCORES - 1) // NCORES  # 62469 windows per core (last core fewer)
BLK = 512
SUPER = 2  # blocks per super-iteration
NSUP = (NPC + SUPER * BLK - 1) // (SUPER * BLK)  # 62
NBLK = NSUP * SUPER  # 124
CAP = NBLK * BLK  # 63488 windows computed per core (incl. padding)
XSH = CAP + 2 * K  # 63744 x-shard length
DSPAN = 4  # super-iterations per diag DMA

_compiled = None


def _build():
    """Build + compile the SPMD Bass program once per process."""
    import concourse.bass as bass
    import concourse.tile as tile
    from concourse import bacc, mybir

    f32 = mybir.dt.float32
    # fp8 e4m3 data path, fp32 PSUM accumulation. Layer 1 runs in
    # DoubleRow perf mode (2 fp8 MACs/cell/cycle, contraction 256 in one
    # matmul); e4m3's +-448 range covers every operand comfortably.
    dt1 = mybir.dt.float8e4
    dt2 = mybir.dt.float8e4
    RELU = mybir.ActivationFunctionType.Relu
    nc = bacc.Bacc("TRN2", target_bir_lowering=False)

    xs = nc.dram_tensor("xs", [XSH], dt1, kind="ExternalInput")
    w1t = nc.dram_tensor("w1t", [128, 256], dt1, kind="ExternalInput")
    w2t = nc.dram_tensor("w2t", [128, 64], dt2, kind="ExternalInput")
    # W3.T duplicated on partition halves so row-tiled layer-3 matmuls can
    # read it from partitions 0-63 and 64-127
    w3t = nc.dram_tensor("w3t", [128, 32], dt2, kind="ExternalInput")
    iout = nc.dram_tensor("iout", [128, NBLK * 4], f32, kind="ExternalOutput")

    DW = SUPER * BLK + 128  # 2176: diag tile width per super-iteration

    with tile.TileContext(nc) as tc:
        with (
            tc.tile_pool(name="const", bufs=1) as cpool,
            tc.tile_pool(name="diag", bufs=2) as dpool,
            tc.tile_pool(name="acts", bufs=2) as rpool,
            tc.tile_pool(name="iacc", bufs=1) as ipool,
            tc.tile_pool(name="psA", bufs=2, space="PSUM") as psA,
            tc.tile_pool(name="psB", bufs=2, space="PSUM") as psB,
            tc.tile_pool(name="psC", bufs=1, space="PSUM") as psC,
        ):
            w1s = cpool.tile([128, 256], dt1)
            nc.sync.dma_start(w1s[:], w1t[:])
            w2s = cpool.tile([128, 64], dt2)
            nc.sync.dma_start(w2s[:], w2t[:])
            w3s = cpool.tile([128, 32], dt2)
            nc.sync.dma_start(w3s[:], w3t[:])
            isb = ipool.tile([128, NBLK * 4], f32)

            SW = SUPER * BLK  # 1024 windows per super-iteration
            # DoubleRow weights AP: contraction = (partition i, ktile q)
            # over taps 128 q + i; free dims [q, elem] with q-step 128
            w1dr = bass.AP(
                w1s[:].tensor, w1s[:].offset,
                [list(w1s[:].ap[0]), [128, 2], [1, 128]],
            )
            for ds in range(0, NSUP, DSPAN):
                span = min(DSPAN, NSUP - ds)
                dw = span * SW + 128
                d = dpool.tile([128, DSPAN * SW + 128], dt1, tag="d")
                nc.sync.dma_start(
                    d[:, :dw], bass.AP(xs, ds * SW, [[1, 128], [1, dw]])
                )
                for s in range(ds, ds + span):
                    off = (s - ds) * SW
                    # layer 1: one DoubleRow matmul per 512-window block
                    p1 = [
                        psA.tile(
                            [128, BLK], f32, name=f"p1_{k}", tag=f"p1_{k}",
                        )
                        for k in range(SUPER)
                    ]
                    for k in range(SUPER):
                        dsl = d[:, off + BLK * k : off + BLK * k + BLK + 128]
                        ddr = bass.AP(
                            dsl.tensor, dsl.offset,
                            [list(dsl.ap[0]), [128, 2], [1, BLK]],
                        )
                        nc.tensor.matmul(
                            p1[k][:], w1dr, ddr,
                            start=True, stop=True,
                            perf_mode=mybir.MatmulPerfMode.DoubleRow,
                        )
                    # relu split across ACT and DVE to balance engine load
                    r1 = []
                    for k in range(SUPER):
                        t = rpool.tile(
                            [128, BLK], dt2, name=f"r1_{k}", tag=f"r1_{k}"
                        )
                        if k == 0:
                            nc.scalar.activation(t[:], p1[k][:], RELU)
                        else:
                            nc.vector.tensor_relu(t[:], p1[k][:])
                        r1.append(t)

                    # layer 2: the block pair packed on array column halves
                    p2 = psB.tile([128, BLK], f32, name="p2", tag="p2")
                    nc.tensor.matmul(
                        p2[0:64, :], w2s[:], r1[0][:],
                        start=True, stop=True, tile_position=(0, 0),
                    )
                    nc.tensor.matmul(
                        p2[64:128, :], w2s[:], r1[1][:],
                        start=True, stop=True, tile_position=(0, 64),
                    )
                    r2 = rpool.tile([128, BLK], dt2, name="r2", tag="r2")
                    nc.scalar.activation(r2[:], p2[:], RELU)

                    # layer 3: stationary = relu(h2) chunks, moving = W3.T;
                    # the two blocks run concurrently on array row halves,
                    # each row group draining into its own PSUM bank
                    p3a = psC.tile([128, 128], f32, name="p3a", tag="p3a")
                    p3b = psC.tile([128, 128], f32, name="p3b", tag="p3b")
                    for c in range(4):
                        nc.tensor.matmul(
                            p3a[:, 32 * c : 32 * c + 32],
                            r2[0:64, 128 * c : 128 * (c + 1)],
                            w3s[0:64, :],
                            start=True, stop=True, tile_position=(0, 0),
                        )
                        nc.tensor.matmul(
                            p3b[:, 32 * c : 32 * c + 32],
                            r2[64:128, 128 * c : 128 * (c + 1)],
                            w3s[64:128, :],
                            start=True, stop=True, tile_position=(64, 0),
                        )
                    # p3a col 32c+k3 <-> window 1024 s + 128 c + p
                    # p3b col 32c+k3 <-> window 1024 s + 512 + 128 c + p
                    nc.vector.tensor_reduce(
                        isb[:, 8 * s : 8 * s + 4],
                        p3a[:].rearrange("p (g k) -> p g k", k=32),
                        axis=mybir.AxisListType.X,
                        op=mybir.AluOpType.max,
                    )
                    nc.vector.tensor_reduce(
                        isb[:, 8 * s + 4 : 8 * s + 8],
                        p3b[:].rearrange("p (g k) -> p g k", k=32),
                        axis=mybir.AxisListType.X,
                        op=mybir.AluOpType.max,
                    )

            nc.sync.dma_start(iout[:], isb[:])

    nc.compile()
    return nc


def _get_compiled():
    global _compiled
    if _compiled is None:
        _compiled = _build()
    return _compiled


def _run_device(x, W1, W2, W3, trace=False):
    """Shard across 8 cores, run, return full pre-activation max array [N]."""
    import ml_dtypes
    from concourse.bass_utils import run_bass_kernel_spmd

    nc = _get_compiled()
    f8a = ml_dtypes.float8_e4m3
    f8b = ml_dtypes.float8_e4m3

    x = np.ascontiguousarray(np.asarray(x, np.float32))
    xpad = np.zeros((NCORES - 1) * NPC + XSH, f8a)
    xpad[:T] = np.clip(x, -448, 448).astype(f8a)
    w1 = np.ascontiguousarray(
        np.clip(np.concatenate([W1.T[:128], W1.T[128:]], axis=1), -448, 448)
        .astype(f8a)
    )  # [128, 256]: [:, :128] = taps 0-127, [:, 128:] = taps 128-255
    w2 = np.ascontiguousarray(W2.T.astype(f8b))  # [128, 64]
    w3 = np.ascontiguousarray(
        np.concatenate([W3.T, W3.T], axis=0).astype(f8b)
    )  # [128, 32] = W3.T stacked twice

    in_maps = [
        {
            "xs": np.ascontiguousarray(xpad[i * NPC : i * NPC + XSH]),
            "w1t": w1,
            "w2t": w2,
            "w3t": w3,
        }
        for i in range(NCORES)
    ]
    res = run_bass_kernel_spmd(
        nc, in_maps, core_ids=list(range(NCORES)), trace=trace
    )

    maxpre = np.empty(N, np.float32)
    for i in range(NCORES):
        arr = res.results[i]["iout"]  # [128, NBLK*4]
        # col = 8 s + 4 par + c; window n = 1024 s + 512 par + 128 c + p
        loc = (
            arr.reshape(128, NSUP, 2, 4)  # p, s, par, c
            .transpose(1, 2, 3, 0)  # s, par, c, p
            .reshape(-1)
        )
        s = i * NPC
        cnt = min(NPC, N - s)
        maxpre[s : s + cnt] = loc[:cnt]
    return maxpre, res


def _host_finish(maxpre, x, W1, W2, W3):
    """Replicate the reference's LIF chain + argmin + winner (f32, host)."""
    f32 = np.float32
    I = (np.maximum(maxpre, 0) * f32(2.0)).astype(f32)
    safe = np.where(
        I > 1.0, f32(1.0) - f32(1.0) / np.maximum(I, f32(1.0 + 1e-12)), f32(0.5)
    ).astype(f32)
    n = np.maximum(np.ceil(np.log(safe) / np.log(f32(DECAY))), f32(1.0)).astype(f32)
    spikes = (I > 1.0) & (n <= MAX_STEPS)
    latency = np.where(spikes, n * f32(DT), f32(np.inf)).astype(f32)
    abs_times = (np.arange(N, dtype=f32) + latency).astype(f32)
    best = int(np.argmin(abs_times))

    # recompute the reported values from the f32 window (matches the
    # reference's f32 chain; device bf16 only picks the argmin window)
    xw = np.asarray(x, f32)[best : best + W_WIN]
    W1f = np.asarray(W1, f32)
    W2f = np.asarray(W2, f32)
    W3f = np.asarray(W3, f32)
    h1 = np.maximum(W1f @ xw, 0)
    h2 = np.maximum(W2f @ h1, 0)
    h3 = np.maximum(W3f @ h2, 0)
    winner = int(np.argmax(h3))

    Ib = f32(h3.max() * f32(2.0))
    safeb = (
        f32(1.0) - f32(1.0) / max(Ib, f32(1.0 + 1e-12)) if Ib > 1.0 else f32(0.5)
    )
    nb = f32(max(np.ceil(np.log(f32(safeb)) / np.log(f32(DECAY))), 1.0))
    spikeb = (Ib > 1.0) and (nb <= MAX_STEPS)
    latb = f32(nb * f32(DT)) if spikeb else f32(np.inf)
    absb = f32(f32(best) + latb)

    return (
        np.int32(best),
        np.int32(winner),
        f32(latb),
        f32(absb),
    )


def kernel(x, W1, W2, W3):
    maxpre, _ = _run_device(x, W1, W2, W3)
    return _host_finish(maxpre, x, W1, W2, W3)
